# revision 3
# baseline (speedup 1.0000x reference)
"""Dense dot-product attention (B=8, S=2048, D=64, fp32) on 8 TRN2 NeuronCores.

Sharding: batch dim across the 8 cores (data parallel), one batch element per
core. v3 design (bf16 matmul path, fp32 softmax/epilogue):

  Host prep: Q/K zero-padded to [S, 128] bf16 (XBAR DMA transpose needs
  free%128==0, 2-byte dtype); V bf16; masks pre-transposed to [128, 16].

  Q^T/K^T land in SBUF via hardware XBAR transpose DMAs (no PE transposes,
  no casts). The key padding mask enters through the exp bias (per-partition
  AP), so the score matmul is a clean 64-row contraction with 128-wide bf16
  weights.

  HAM warmup: the PE clock-gate defaults to 1.2 GHz and only reaches 2.4 GHz
  after ~3.4us of sustained matmul activity; a cold main loop is a stable
  bad equilibrium (PE dense-but-cold, ACT waiting). A dependency-free burst
  of dummy matmuls during the DMA window un-throttles the PE before the
  first real ST matmul, and the warm loop then sustains itself (PE work per
  tile ~1.06us < exp 1.11us).

  Main loop per k-chunk n (128 rows), per q-halftile e (1024 cols):
    ST[k,q] = K_chunk @ Q^T (2 x 512 matmuls, bf16)
    se = exp(0.125*ST + maskbias)  on ACT (the 35.5us roofline), bf16
    PV[d,q] += V'_chunk^T @ se     (V' = [v*mask_v | 1 | 0pad], 128 wide)

  Epilogue in 4 pipelined column groups: pv->SBUF copy (ACT/DVE alternating),
  4 PE transposes, strided reciprocal, scaled multiplies (DVE/ACT), and a
  quarter output DMA per group on alternating queues.
"""

import numpy as np
import ml_dtypes

import concourse.bass as bass
import concourse.mybir as mybir
import concourse.tile as tile
from concourse import bacc
from concourse.bass import ts
from concourse.bass_utils import run_bass_kernel_spmd
from concourse.masks import make_identity

B, S, D = 8, 2048, 64
NEG = -1e9
P = 128
NKC = S // P     # 16 k-chunks
EW = 1024        # exp granularity (q width per ST tile)
NE = S // EW     # ST tiles per chunk
MMW = 512        # matmul moving width (one fp32 PSUM bank)
WU = 14          # HAM warmup matmuls
F32 = mybir.dt.float32
BF16 = mybir.dt.bfloat16
BF = ml_dtypes.bfloat16

_CACHE: dict = {}


def _build_nc():
    nc = bacc.Bacc("TRN2", target_bir_lowering=False, debug=False)

    qp = nc.dram_tensor("qp", [S, P], BF16, kind="ExternalInput").ap()
    kp = nc.dram_tensor("kp", [S, P], BF16, kind="ExternalInput").ap()
    vn = nc.dram_tensor("vn", [S, D], BF16, kind="ExternalInput").ap()
    mkt = nc.dram_tensor("mkt", [P, NKC], F32, kind="ExternalInput").ap()
    mvt = nc.dram_tensor("mvt", [P, NKC], F32, kind="ExternalInput").ap()
    out = nc.dram_tensor("out", [S, D], F32, kind="ExternalOutput").ap()

    with tile.TileContext(nc) as tc:
        with (
            tc.tile_pool(name="const", bufs=1) as const,
            tc.tile_pool(name="se", bufs=3) as se_pool,
        ):
            ident = const.tile([P, P], F32)
            make_identity(nc, ident)

            qt = const.tile([P, S], BF16, tag="qt")
            kt = const.tile([P, S], BF16, tag="kt")
            vf = const.tile([P, NKC, D], BF16, tag="vf")
            vp = const.tile([P, NKC, P], BF16, tag="vp")
            mks = const.tile([P, NKC], F32, tag="mks")
            mkb = const.tile([P, NKC], F32, tag="mkb")
            mvs = const.tile([P, NKC], F32, tag="mvs")
            ob = const.tile([P, NKC, D], F32, tag="ob")
            pvsb = const.tile([D + 1, S], F32, tag="pvsb")
            recs = const.tile([P, NKC], F32, tag="recs")
            wl = const.tile([D, MMW], BF16, tag="wl")

            # HAM warmup: dependency-free dense matmuls so the PE clock-gate
            # opens (1.2 -> 2.4 GHz) while the input DMAs are in flight.
            nc.gpsimd.memset(wl, 1.0)
            with tc.tile_pool(name="wu_ps", bufs=1, space="PSUM") as wu_ps:
                wps = wu_ps.tile([P, MMW], F32, tag="wps")
                for _ in range(WU):
                    nc.tensor.matmul(
                        wps, lhsT=wl[:, 0:P], rhs=wl, start=True, stop=True
                    )

            # Input DMAs: masks are tiny host-transposed [128,16] tensors
            # (contiguous per partition). k/masks on sync queue, q/v on the
            # scalar queue so descriptor generation and transfers overlap.
            HS = S // 2
            nc.sync.dma_start(out=mks, in_=mkt)
            nc.scalar.dma_start_transpose(out=qt[:, 0:HS], in_=qp[0:HS, :])
            nc.sync.dma_start_transpose(out=kt[:, 0:HS], in_=kp[0:HS, :])
            nc.scalar.dma_start(out=vf, in_=vn.rearrange("(n p) d -> p n d", p=P))
            nc.sync.dma_start_transpose(out=kt[:, HS:S], in_=kp[HS:S, :])
            nc.scalar.dma_start_transpose(out=qt[:, HS:S], in_=qp[HS:S, :])
            nc.sync.dma_start(out=mvs, in_=mvt)

            # Key-mask additive bias rides in the exp: bias = (mk-1)*1e9.
            nc.gpsimd.tensor_scalar(
                mkb, mks, -NEG, NEG,
                op0=mybir.AluOpType.mult, op1=mybir.AluOpType.add,
            )

            # V' chunks: [128, 128] with cols 0:64 = V*mask_v, col 64 = 1.0,
            # cols 65:128 = 0 (128-wide weights -> fast weight load; the
            # extra pv output partitions are never read).
            nc.gpsimd.memset(vp[:, :, D + 1 : P], 0.0)
            nc.gpsimd.memset(vp[:, :, D : D + 1], 1.0)
            for n in range(NKC):
                nc.vector.tensor_scalar(
                    vp[:, n, 0:D], vf[:, n, :], mvs[:, n : n + 1], None,
                    op0=mybir.AluOpType.mult,
                )

            # Main loop: ST tile -> exp -> PV accumulate
            with tc.tile_pool(name="pv_ps", bufs=1, space="PSUM") as pv_ps:
                pv = pv_ps.tile([P, S], F32, tag="pv")
                with tc.tile_pool(name="st_ps", bufs=2, space="PSUM") as st_ps:
                    for n in range(NKC):
                        for e in range(NE):
                            st = st_ps.tile([P, EW], F32, tag="st")
                            for h in range(EW // MMW):
                                nc.tensor.matmul(
                                    st[:, ts(h, MMW)],
                                    lhsT=kt[0:D, ts(n, P)],
                                    rhs=qt[0:D, ts(e * (EW // MMW) + h, MMW)],
                                    start=True,
                                    stop=True,
                                )
                            se = se_pool.tile([P, EW], BF16, tag="se")
                            nc.scalar.activation(
                                se, st, mybir.ActivationFunctionType.Exp,
                                bias=mkb[:, n : n + 1], scale=0.125,
                            )
                            for h in range(EW // MMW):
                                nc.tensor.matmul(
                                    pv[:, ts(e * (EW // MMW) + h, MMW)],
                                    lhsT=vp[:, n, :],
                                    rhs=se[:, ts(h, MMW)],
                                    start=(n == 0),
                                    stop=(n == NKC - 1),
                                )

                # Epilogue: 4 pipelined groups of 4 q-subtiles each.
                with tc.tile_pool(name="ep_ps", bufs=1, space="PSUM") as ep_ps:
                    otall = ep_ps.tile([P, NKC, P], F32, tag="ot")
                    orr = out.rearrange("(n p) d -> p n d", p=P)
                    GW = S // 4  # 512 cols per group, 4 m-subtiles
                    for g in range(4):
                        sl = slice(g * GW, (g + 1) * GW)
                        if g % 2 == 0:
                            nc.scalar.copy(pvsb[:, sl], pv[0 : D + 1, sl])
                        else:
                            nc.vector.tensor_copy(pvsb[:, sl], pv[0 : D + 1, sl])
                        for m in range(4 * g, 4 * g + 4):
                            nc.tensor.transpose(
                                otall[:, m, 0 : D + 1],
                                pvsb[:, ts(m, P)],
                                ident[0 : D + 1, 0 : D + 1],
                            )
                        ms = slice(4 * g, 4 * g + 4)
                        nc.vector.reciprocal(recs[:, ms], otall[:, ms, D])
                        for m in range(4 * g, 4 * g + 4):
                            if m % 2 == 0:
                                nc.vector.tensor_scalar(
                                    ob[:, m, :], otall[:, m, 0:D],
                                    recs[:, m : m + 1], None,
                                    op0=mybir.AluOpType.mult,
                                )
                            else:
                                nc.scalar.mul(
                                    ob[:, m, :], otall[:, m, 0:D],
                                    recs[:, m : m + 1],
                                )
                        eng = nc.sync if g % 2 == 0 else nc.scalar
                        eng.dma_start(out=orr[:, ms, :], in_=ob[:, ms, :])

    nc.compile()
    return nc


def get_nc():
    if "nc" not in _CACHE:
        _CACHE["nc"] = _build_nc()
    return _CACHE["nc"]


def _in_maps(queries, keys, values, mask_k, mask_v):
    qpad = np.zeros((B, S, P), dtype=np.float32)
    qpad[:, :, 0:D] = queries
    kpad = np.zeros((B, S, P), dtype=np.float32)
    kpad[:, :, 0:D] = keys
    qpb = qpad.astype(BF)
    kpb = kpad.astype(BF)
    vb = np.asarray(values, dtype=np.float32).astype(BF)
    # masks pre-transposed to the on-chip [128 partitions, 16 chunks] layout
    mktn = np.ascontiguousarray(
        np.asarray(mask_k, dtype=np.float32).reshape(B, NKC, P).transpose(0, 2, 1)
    )
    mvtn = np.ascontiguousarray(
        np.asarray(mask_v, dtype=np.float32).reshape(B, NKC, P).transpose(0, 2, 1)
    )
    return [
        {
            "qp": np.ascontiguousarray(qpb[b]),
            "kp": np.ascontiguousarray(kpb[b]),
            "vn": np.ascontiguousarray(vb[b]),
            "mkt": mktn[b],
            "mvt": mvtn[b],
        }
        for b in range(B)
    ]


def kernel(queries, keys, values, mask_q, mask_k, mask_v, **_unused):
    nc = get_nc()
    in_maps = _in_maps(queries, keys, values, mask_k, mask_v)
    res = run_bass_kernel_spmd(nc, in_maps, core_ids=list(range(B)))
    return np.stack([res.results[b]["out"] for b in range(B)], axis=0)


# revision 11
# speedup vs baseline: 1.3018x; 1.3018x over previous
"""Dense dot-product attention (B=8, S=2048, D=64, fp32) on 8 TRN2 NeuronCores.

Sharding: batch dim across the 8 cores (data parallel), one batch element per
core. v4 design (f32r matmuls - bf16 provably never opens the HAM clock gate
on this part, f32r streams at 2.4 GHz):

  Layouts: QT/KT = [64, S] f32r (head-dim on partitions, via PE transpose +
  DVE cast), V' = [S, 128] f32r ([v*mask_v | ones | zero-pad]).

  The key padding mask no longer needs a 65th contraction row: it rides in
  the exp's per-partition bias AP (bias = (mask_k-1)*1e9, host-transposed to
  the [128, 16] chunk layout). The ACT engine therefore does NOTHING in the
  main loop except the 32 exps - its 35.5us is the loop roofline. All
  PSUM->SBUF casts go to the DVE.

  Main loop runs e-major (all 16 chunks at q-cols 0:1024, then cols
  1024:2048) so only q-chunks 0..7 gate the first exp. Epilogue groups 0/1
  (cols 0:1024) run DVE-only, interleaved into pass 2 while ACT exps; only
  groups 2/3 trail the last exp.
"""

import numpy as np

import concourse.bass as bass
import concourse.mybir as mybir
import concourse.tile as tile
from concourse import bacc
from concourse.bass import ts
from concourse.bass_utils import run_bass_kernel_spmd
from concourse.masks import make_identity

B, S, D = 8, 2048, 64
NEG = -1e9
P = 128
NKC = S // P     # 16 k-chunks
EW = 1024        # exp granularity (q width per ST tile)
NE = S // EW     # 2 q passes
MMW = 512        # matmul moving width (one fp32 PSUM bank)
F32 = mybir.dt.float32
F32R = mybir.dt.float32r
_CACHE: dict = {}


def _build_nc():
    nc = bacc.Bacc("TRN2", target_bir_lowering=False, debug=False)

    q = nc.dram_tensor("q", [S, D], F32, kind="ExternalInput").ap()
    k = nc.dram_tensor("k", [S, D], F32, kind="ExternalInput").ap()
    v = nc.dram_tensor("v", [S, D], F32, kind="ExternalInput").ap()
    mkt = nc.dram_tensor("mkt", [P, NKC], F32, kind="ExternalInput").ap()
    mvt = nc.dram_tensor("mvt", [P, NKC], F32, kind="ExternalInput").ap()
    out = nc.dram_tensor("out", [S, D], F32, kind="ExternalOutput").ap()

    with tile.TileContext(nc) as tc:
        with (
            tc.tile_pool(name="const", bufs=1) as const,
            tc.tile_pool(name="se", bufs=3) as se_pool,
        ):
            ident = const.tile([P, P], F32)
            make_identity(nc, ident)

            qt = const.tile([D, S], F32R, tag="qt")
            kt = const.tile([D, S], F32R, tag="kt")
            qf = const.tile([P, NKC, D], F32, tag="qf")
            kf = const.tile([P, NKC, D], F32, tag="kf")
            vf = const.tile([P, NKC, D], F32, tag="vf")
            vp = const.tile([P, NKC, P], F32R, tag="vp")
            mks = const.tile([P, NKC], F32, tag="mks")
            mkb = const.tile([P, NKC], F32, tag="mkb")
            mvs = const.tile([P, NKC], F32, tag="mvs")
            ob = const.tile([P, NKC, D], F32, tag="ob")
            pvsb = const.tile([D + 1, S], F32, tag="pvsb")
            recs = const.tile([P, NKC], F32, tag="recs")

            # Input DMAs, all on the sync queue (mixing queues / XBAR
            # transposes made the tile scheduler serialize transfers).
            # Two slices per tensor so chunk 0 lands early.
            qr = q.rearrange("(n p) d -> p n d", p=P)
            kr = k.rearrange("(n p) d -> p n d", p=P)
            vr = v.rearrange("(n p) d -> p n d", p=P)
            H = NKC // 2
            s0, s1 = slice(0, H), slice(H, NKC)
            nc.sync.dma_start(out=qf[:, s0, :], in_=qr[:, s0, :])
            nc.sync.dma_start(out=kf[:, s0, :], in_=kr[:, s0, :])
            nc.sync.dma_start(out=mks, in_=mkt)
            nc.sync.dma_start(out=vf[:, s0, :], in_=vr[:, s0, :])
            nc.sync.dma_start(out=mvs, in_=mvt)
            nc.sync.dma_start(out=qf[:, s1, :], in_=qr[:, s1, :])
            nc.sync.dma_start(out=kf[:, s1, :], in_=kr[:, s1, :])
            nc.sync.dma_start(out=vf[:, s1, :], in_=vr[:, s1, :])

            # Key-mask additive bias rides in the exp: bias = (mk-1)*1e9.
            nc.gpsimd.tensor_scalar(
                mkb, mks, -NEG, NEG,
                op0=mybir.AluOpType.mult, op1=mybir.AluOpType.add,
            )

            # V' chunks: [128, 128]: cols 0:64 = V*mask_v, col 64 = 1.0
            # (denominator row), cols 65:128 = 0.
            # memset through an f32 view: 0.0/1.0 have identical f32/f32r
            # bits, and the BIR verifier rejects unrounded f32r producers.
            nc.gpsimd.memset(vp[:, :, D + 1 : P].bitcast(F32), 0.0)
            nc.gpsimd.memset(vp[:, :, D : D + 1].bitcast(F32), 1.0)
            for n in range(NKC):
                nc.vector.tensor_scalar(
                    vp[:, n, 0:D], vf[:, n, :], mvs[:, n : n + 1], None,
                    op0=mybir.AluOpType.mult,
                )

            # Q^T / K^T via PE transpose + DVE cast out. Emission order =
            # first-ST dependency order; the cold->warm transpose stream is
            # also the HAM warmup.
            tp_order = [("q", i) for i in range(4)] + [("k", 0)]
            tp_order += [("q", i) for i in range(4, 8)] + [("k", 1)]
            rest_k = list(range(2, NKC))
            rest_q = list(range(8, NKC))
            for i, n in enumerate(rest_k):
                tp_order.append(("k", n))
                if i < len(rest_q):
                    tp_order.append(("q", rest_q[i]))
            with tc.tile_pool(name="tp_ps", bufs=4, space="PSUM") as tp_ps:
                for which, n in tp_order:
                    tp = tp_ps.tile([D, P], F32, tag="tps")
                    nc.tensor.transpose(tp, (qf if which == "q" else kf)[:, n, :], ident)
                    dst = qt if which == "q" else kt
                    nc.vector.tensor_copy(dst[:, ts(n, P)], tp)

            def ep_copy(g, on_act):
                """pv cols [512g, 512g+512) -> SBUF staging (no PSUM needed,
                so groups 0/1 can run during pass 2 while ACT exps)."""
                sl = slice(g * MMW, (g + 1) * MMW)
                if on_act:
                    nc.scalar.copy(pvsb[:, sl], pv[0 : D + 1, sl])
                else:
                    nc.vector.tensor_copy(pvsb[:, sl], pv[0 : D + 1, sl])

            def ep_finish(g):
                """4 transposes, strided reciprocal, scaled mults, quarter
                output DMA for q-cols [512g, 512g+512)."""
                ms = slice(4 * g, 4 * g + 4)
                for m in range(4 * g, 4 * g + 4):
                    nc.tensor.transpose(
                        otall[:, m, 0 : D + 1],
                        pvsb[:, ts(m, P)],
                        ident[0 : D + 1, 0 : D + 1],
                    )
                nc.vector.reciprocal(recs[:, ms], otall[:, ms, D])
                for m in range(4 * g, 4 * g + 4):
                    if m % 2 == 0:
                        nc.vector.tensor_scalar(
                            ob[:, m, :], otall[:, m, 0:D],
                            recs[:, m : m + 1], None,
                            op0=mybir.AluOpType.mult,
                        )
                    else:
                        nc.scalar.mul(
                            ob[:, m, :], otall[:, m, 0:D], recs[:, m : m + 1]
                        )
                eng = nc.sync if g % 2 == 0 else nc.scalar
                eng.dma_start(out=orr[:, ms, :], in_=ob[:, ms, :])

            orr = out.rearrange("(n p) d -> p n d", p=P)

            # Main loop, e-major: pass 0 = q cols 0:1024, pass 1 = rest.
            with tc.tile_pool(name="pv_ps", bufs=1, space="PSUM") as pv_ps:
                pv = pv_ps.tile([P, S], F32, tag="pv")
                otall = None
                with tc.tile_pool(name="st_ps", bufs=2, space="PSUM") as st_ps:
                    for e in range(NE):
                        for n in range(NKC):
                            st = st_ps.tile([P, EW], F32, tag="st")
                            for h in range(EW // MMW):
                                nc.tensor.matmul(
                                    st[:, ts(h, MMW)],
                                    lhsT=kt[:, ts(n, P)],
                                    rhs=qt[:, ts(e * (EW // MMW) + h, MMW)],
                                    start=True,
                                    stop=True,
                                )
                            se = se_pool.tile([P, EW], F32R, tag="se")
                            nc.scalar.activation(
                                se, st, mybir.ActivationFunctionType.Exp,
                                bias=mkb[:, n : n + 1], scale=0.125,
                            )
                            for h in range(EW // MMW):
                                nc.tensor.matmul(
                                    pv[:, ts(e * (EW // MMW) + h, MMW)],
                                    lhsT=vp[:, n, :],
                                    rhs=se[:, ts(h, MMW)],
                                    start=(n == 0),
                                    stop=(n == NKC - 1),
                                )
                            if e == 1 and n == 5:
                                ep_copy(0, on_act=False)
                            if e == 1 and n == 11:
                                ep_copy(1, on_act=False)

                # st pool closed: its 4 banks are free for otall.
                with tc.tile_pool(name="ep_ps", bufs=1, space="PSUM") as ep_ps:
                    otall = ep_ps.tile([P, NKC, P], F32, tag="ot")
                    ep_copy(2, on_act=True)
                    ep_copy(3, on_act=False)
                    for g in range(4):
                        ep_finish(g)

    nc.compile()
    return nc


def get_nc():
    if "nc" not in _CACHE:
        _CACHE["nc"] = _build_nc()
    return _CACHE["nc"]


def _in_maps(queries, keys, values, mask_k, mask_v):
    mktn = np.ascontiguousarray(
        np.asarray(mask_k, dtype=np.float32).reshape(B, NKC, P).transpose(0, 2, 1)
    )
    mvtn = np.ascontiguousarray(
        np.asarray(mask_v, dtype=np.float32).reshape(B, NKC, P).transpose(0, 2, 1)
    )
    return [
        {
            "q": np.ascontiguousarray(queries[b], dtype=np.float32),
            "k": np.ascontiguousarray(keys[b], dtype=np.float32),
            "v": np.ascontiguousarray(values[b], dtype=np.float32),
            "mkt": mktn[b],
            "mvt": mvtn[b],
        }
        for b in range(B)
    ]


def kernel(queries, keys, values, mask_q, mask_k, mask_v, **_unused):
    nc = get_nc()
    in_maps = _in_maps(queries, keys, values, mask_k, mask_v)
    res = run_bass_kernel_spmd(nc, in_maps, core_ids=list(range(B)))
    return np.stack([res.results[b]["out"] for b in range(B)], axis=0)


# revision 12
# speedup vs baseline: 1.3250x; 1.0178x over previous
"""Dense dot-product attention (B=8, S=2048, D=64, fp32) on 8 TRN2 NeuronCores.

Sharding: batch dim across the 8 cores (data parallel), one batch element per
core. v4 design (f32r matmuls - bf16 provably never opens the HAM clock gate
on this part, f32r streams at 2.4 GHz):

  Layouts: QT/KT = [64, S] f32r (head-dim on partitions, via PE transpose +
  DVE cast), V' = [S, 128] f32r ([v*mask_v | ones | zero-pad]).

  The key padding mask no longer needs a 65th contraction row: it rides in
  the exp's per-partition bias AP (bias = (mask_k-1)*1e9, host-transposed to
  the [128, 16] chunk layout). The ACT engine therefore does NOTHING in the
  main loop except the 32 exps - its 35.5us is the loop roofline. All
  PSUM->SBUF casts go to the DVE.

  Main loop runs e-major (all 16 chunks at q-cols 0:1024, then cols
  1024:2048) so only q-chunks 0..7 gate the first exp. Epilogue groups 0/1
  (cols 0:1024) run DVE-only, interleaved into pass 2 while ACT exps; only
  groups 2/3 trail the last exp.
"""

import numpy as np

import concourse.bass as bass
import concourse.mybir as mybir
import concourse.tile as tile
from concourse import bacc
from concourse.bass import ts
from concourse.bass_utils import run_bass_kernel_spmd
from concourse.masks import make_identity

B, S, D = 8, 2048, 64
NEG = -1e9
P = 128
NKC = S // P     # 16 k-chunks
EW = 1024        # exp granularity (q width per ST tile)
NE = S // EW     # 2 q passes
MMW = 512        # matmul moving width (one fp32 PSUM bank)
F32 = mybir.dt.float32
F32R = mybir.dt.float32r
_CACHE: dict = {}


def _build_nc():
    nc = bacc.Bacc("TRN2", target_bir_lowering=False, debug=False)

    q = nc.dram_tensor("q", [S, D], F32, kind="ExternalInput").ap()
    k = nc.dram_tensor("k", [S, D], F32, kind="ExternalInput").ap()
    v = nc.dram_tensor("v", [S, D], F32, kind="ExternalInput").ap()
    mkt = nc.dram_tensor("mkt", [P, NKC], F32, kind="ExternalInput").ap()
    mvt = nc.dram_tensor("mvt", [P, NKC], F32, kind="ExternalInput").ap()
    out = nc.dram_tensor("out", [S, D], F32, kind="ExternalOutput").ap()

    with tile.TileContext(nc) as tc:
        with (
            tc.tile_pool(name="const", bufs=1) as const,
            tc.tile_pool(name="se", bufs=3) as se_pool,
        ):
            ident = const.tile([P, P], F32)
            make_identity(nc, ident)

            qt = const.tile([D, S], F32R, tag="qt")
            kt = const.tile([D, S], F32R, tag="kt")
            qf = const.tile([P, NKC, D], F32, tag="qf")
            kf = const.tile([P, NKC, D], F32, tag="kf")
            vf = const.tile([P, NKC, D], F32, tag="vf")
            vp = const.tile([P, NKC, D + 1], F32R, tag="vp")
            mks = const.tile([P, NKC], F32, tag="mks")
            mkb = const.tile([P, NKC], F32, tag="mkb")
            mvs = const.tile([P, NKC], F32, tag="mvs")
            ob = const.tile([P, NKC, D], F32, tag="ob")
            pvsb = const.tile([D + 1, S], F32, tag="pvsb")
            recs = const.tile([P, NKC], F32, tag="recs")

            # Input DMAs, all on the sync queue (mixing queues / XBAR
            # transposes made the tile scheduler serialize transfers).
            # Two slices per tensor so chunk 0 lands early.
            qr = q.rearrange("(n p) d -> p n d", p=P)
            kr = k.rearrange("(n p) d -> p n d", p=P)
            vr = v.rearrange("(n p) d -> p n d", p=P)
            H = NKC // 2
            s0, s1 = slice(0, H), slice(H, NKC)
            nc.sync.dma_start(out=qf[:, s0, :], in_=qr[:, s0, :])
            nc.sync.dma_start(out=kf[:, s0, :], in_=kr[:, s0, :])
            nc.sync.dma_start(out=mks, in_=mkt)
            nc.sync.dma_start(out=vf[:, s0, :], in_=vr[:, s0, :])
            nc.sync.dma_start(out=mvs, in_=mvt)
            nc.sync.dma_start(out=qf[:, s1, :], in_=qr[:, s1, :])
            nc.sync.dma_start(out=kf[:, s1, :], in_=kr[:, s1, :])
            nc.sync.dma_start(out=vf[:, s1, :], in_=vr[:, s1, :])

            # Key-mask additive bias rides in the exp: bias = (mk-1)*1e9.
            nc.gpsimd.tensor_scalar(
                mkb, mks, -NEG, NEG,
                op0=mybir.AluOpType.mult, op1=mybir.AluOpType.add,
            )

            # V' chunks: [128, 65]: cols 0:64 = V*mask_v, col 64 = 1.0
            # (denominator row). All on GpSimd: the DVE queue head must stay
            # free for the qt/kt casts that gate the exp stream (memset via
            # f32 view - the BIR verifier rejects unrounded f32r producers).
            nc.gpsimd.memset(vp[:, :, D : D + 1].bitcast(F32), 1.0)
            for n in range(NKC):
                nc.gpsimd.tensor_scalar(
                    vp[:, n, 0:D], vf[:, n, :], mvs[:, n : n + 1], None,
                    op0=mybir.AluOpType.mult,
                )

            # Q^T / K^T via PE transpose + DVE cast out. Emission order =
            # first-ST dependency order; the cold->warm transpose stream is
            # also the HAM warmup.
            tp_order = [("q", i) for i in range(4)] + [("k", 0)]
            tp_order += [("q", i) for i in range(4, 8)] + [("k", 1)]
            rest_k = list(range(2, NKC))
            rest_q = list(range(8, NKC))
            for i, n in enumerate(rest_k):
                tp_order.append(("k", n))
                if i < len(rest_q):
                    tp_order.append(("q", rest_q[i]))
            with tc.tile_pool(name="tp_ps", bufs=4, space="PSUM") as tp_ps:
                for which, n in tp_order:
                    tp = tp_ps.tile([D, P], F32, tag="tps")
                    nc.tensor.transpose(tp, (qf if which == "q" else kf)[:, n, :], ident)
                    dst = qt if which == "q" else kt
                    nc.vector.tensor_copy(dst[:, ts(n, P)], tp)

            def ep_copy(g, on_act):
                """pv cols [512g, 512g+512) -> SBUF staging (no PSUM needed,
                so groups 0/1 can run during pass 2 while ACT exps)."""
                sl = slice(g * MMW, (g + 1) * MMW)
                if on_act:
                    nc.scalar.copy(pvsb[:, sl], pv[0 : D + 1, sl])
                else:
                    nc.vector.tensor_copy(pvsb[:, sl], pv[0 : D + 1, sl])

            def ep_finish(g):
                """4 transposes, strided reciprocal, scaled mults, quarter
                output DMA for q-cols [512g, 512g+512)."""
                ms = slice(4 * g, 4 * g + 4)
                for m in range(4 * g, 4 * g + 4):
                    nc.tensor.transpose(
                        otall[:, m, 0 : D + 1],
                        pvsb[:, ts(m, P)],
                        ident[0 : D + 1, 0 : D + 1],
                    )
                nc.vector.reciprocal(recs[:, ms], otall[:, ms, D])
                for m in range(4 * g, 4 * g + 4):
                    if m % 2 == 0:
                        nc.vector.tensor_scalar(
                            ob[:, m, :], otall[:, m, 0:D],
                            recs[:, m : m + 1], None,
                            op0=mybir.AluOpType.mult,
                        )
                    else:
                        nc.scalar.mul(
                            ob[:, m, :], otall[:, m, 0:D], recs[:, m : m + 1]
                        )
                nc.sync.dma_start(out=orr[:, ms, :], in_=ob[:, ms, :])

            orr = out.rearrange("(n p) d -> p n d", p=P)

            # Main loop, e-major: pass 0 = q cols 0:1024, pass 1 = rest.
            with tc.tile_pool(name="pv_ps", bufs=1, space="PSUM") as pv_ps:
                pv = pv_ps.tile([D + 1, S], F32, tag="pv")
                otall = None
                with tc.tile_pool(name="st_ps", bufs=2, space="PSUM") as st_ps:
                    for e in range(NE):
                        for n in range(NKC):
                            st = st_ps.tile([P, EW], F32, tag="st")
                            for h in range(EW // MMW):
                                nc.tensor.matmul(
                                    st[:, ts(h, MMW)],
                                    lhsT=kt[:, ts(n, P)],
                                    rhs=qt[:, ts(e * (EW // MMW) + h, MMW)],
                                    start=True,
                                    stop=True,
                                )
                            se = se_pool.tile([P, EW], F32R, tag="se")
                            nc.scalar.activation(
                                se, st, mybir.ActivationFunctionType.Exp,
                                bias=mkb[:, n : n + 1], scale=0.125,
                            )
                            for h in range(EW // MMW):
                                nc.tensor.matmul(
                                    pv[:, ts(e * (EW // MMW) + h, MMW)],
                                    lhsT=vp[:, n, :],
                                    rhs=se[:, ts(h, MMW)],
                                    start=(n == 0),
                                    stop=(n == NKC - 1),
                                )
                            if e == 1 and n == 5:
                                ep_copy(0, on_act=False)
                            if e == 1 and n == 11:
                                ep_copy(1, on_act=False)

                # st pool closed: its 4 banks are free for otall.
                with tc.tile_pool(name="ep_ps", bufs=1, space="PSUM") as ep_ps:
                    otall = ep_ps.tile([P, NKC, P], F32, tag="ot")
                    ep_copy(2, on_act=True)
                    ep_copy(3, on_act=False)
                    for g in range(4):
                        ep_finish(g)

    nc.compile()
    return nc


def get_nc():
    if "nc" not in _CACHE:
        _CACHE["nc"] = _build_nc()
    return _CACHE["nc"]


def _in_maps(queries, keys, values, mask_k, mask_v):
    mktn = np.ascontiguousarray(
        np.asarray(mask_k, dtype=np.float32).reshape(B, NKC, P).transpose(0, 2, 1)
    )
    mvtn = np.ascontiguousarray(
        np.asarray(mask_v, dtype=np.float32).reshape(B, NKC, P).transpose(0, 2, 1)
    )
    return [
        {
            "q": np.ascontiguousarray(queries[b], dtype=np.float32),
            "k": np.ascontiguousarray(keys[b], dtype=np.float32),
            "v": np.ascontiguousarray(values[b], dtype=np.float32),
            "mkt": mktn[b],
            "mvt": mvtn[b],
        }
        for b in range(B)
    ]


def kernel(queries, keys, values, mask_q, mask_k, mask_v, **_unused):
    nc = get_nc()
    in_maps = _in_maps(queries, keys, values, mask_k, mask_v)
    res = run_bass_kernel_spmd(nc, in_maps, core_ids=list(range(B)))
    return np.stack([res.results[b]["out"] for b in range(B)], axis=0)
